# revision 25
# baseline (speedup 1.0000x reference)
"""Distributed GQA attention-with-cache kernel for 8 TRN2 NeuronCores.

Tensor-parallel over heads: core c owns q-heads [4c, 4c+4) and kv-head c.

v2 pipeline (vs the 266us baseline):
- Every bulk DMA reads a host-prearranged FLAT [128, N] image (contiguous
  per-partition lines) so HWDGE descriptor counts drop ~8x and the issuing
  engine isn't serialized behind descriptor generation.
- K streams on the sync ring, V + weights on the scalar ring, small/irregular
  transfers on gpsimd (SWDGE); the scalar engine keeps only the exp
  activations so they are never stuck behind multi-us DMA issues.
- Scores and attn@V are processed in T-halves: scores(half1) overlaps AV
  attn@V(half0), keeping both the DMA stream and the PE continuously busy
  (HAM stays warm).  AV accumulates half0 in SBUF, folds half1 + the
  new-token block + 1/rowsum at the end.
- Score matmuls for the 4 heads are column-tiled to PSUM quarters
  (tile_position) so the 4 tiny M=4 matmuls run concurrently.
- wo / wkv are prefetched mid-stream; the output projection consumes the
  AllGather result in 32 flat chunks.
"""
import numpy as np
import ml_dtypes

import concourse.bass as bass  # noqa: F401
import concourse.mybir as mybir
import concourse.tile as tile
from concourse import bacc
from concourse.bass_utils import run_bass_kernel_spmd
from concourse.masks import make_identity

# If BASS_TRACE is set but the axon NTFF hook module is absent, bass_utils
# would fail on import; provide a no-op stub so tracing degrades gracefully.
try:
    import antenv.axon_hooks  # noqa: F401
except Exception:
    import sys as _sys
    import types as _types

    _m = _types.ModuleType("antenv.axon_hooks")
    _m.get_axon_ntff_profile_hook = lambda: None
    _m.set_axon_ntff_profile_hook = lambda h: None
    _sys.modules["antenv.axon_hooks"] = _m

B, S, T, L, NH, NKV, HD, DIM = 8, 4, 2048, 2, 32, 8, 128, 4096
N_CORES = 8
HPC = NH // N_CORES          # 4 q-heads per core
CW = HPC * HD                # 512 attn feature cols per core
NTOK = B * S                 # 32 tokens
ROWS = B * HPC * S           # 128 = (h, b, s) rows
TH = T // 2                  # 1024 cache cols per half
NEG = -1.0e30

F32 = mybir.dt.float32
BF16 = mybir.dt.bfloat16
AF = mybir.ActivationFunctionType
ALU = mybir.AluOpType

_CACHE = {}
DEBUG_TAPS = False


def _build():
    ndc = DIM // 128         # 32 contraction chunks

    nc = bacc.Bacc("TRN2", target_bir_lowering=False, debug=False, num_devices=N_CORES)
    xT = nc.declare_dram_parameter("xT", [128, ndc * NTOK], BF16, isOutput=False)
    wqF = nc.declare_dram_parameter("wqF", [4, 128, 8 * CW], BF16, isOutput=False)
    wkvF = nc.declare_dram_parameter("wkvF", [2, 128, 16 * 256], BF16, isOutput=False)
    woF = nc.declare_dram_parameter("woF", [128, ndc * CW], BF16, isOutput=False)
    kF = nc.declare_dram_parameter("kF", [2 * B, 128, HPC * TH], BF16, isOutput=False)
    vF = nc.declare_dram_parameter("vF", [2 * B, 128, 8 * CW], BF16, isOutput=False)
    cosq = nc.declare_dram_parameter("cosq", [NTOK, HPC * HD // 2], F32, isOutput=False)
    sinq = nc.declare_dram_parameter("sinq", [NTOK, HPC * HD // 2], F32, isOutput=False)
    cosk = nc.declare_dram_parameter("cosk", [NTOK, HD // 2], F32, isOutput=False)
    sink = nc.declare_dram_parameter("sink", [NTOK, HD // 2], F32, isOutput=False)
    maskP = nc.declare_dram_parameter("maskP", [ROWS, T], BF16, isOutput=False)
    penApp = nc.declare_dram_parameter("penApp", [ROWS, NTOK], F32, isOutput=False)
    out = nc.declare_dram_parameter("out", [NTOK, CW], F32, isOutput=True)

    with tile.TileContext(nc) as tc:
        with (
            tc.tile_pool(name="const", bufs=1) as cn,
            tc.tile_pool(name="kpool", bufs=8) as kp,
            tc.tile_pool(name="vpool", bufs=9) as vp,
            tc.tile_pool(name="stage", bufs=4) as st,
            tc.tile_pool(name="avsb", bufs=4) as avp,
            tc.tile_pool(name="dram", bufs=1, space="DRAM") as dr,
        ):
            # ---------------- head DMA issues ----------------------------
            xT_sb = cn.tile([128, ndc * NTOK], BF16)
            nc.sync.dma_start(xT_sb[:], xT[:])
            cq = cn.tile([NTOK, HPC * HD // 2], F32)
            sq = cn.tile([NTOK, HPC * HD // 2], F32)
            ck = cn.tile([NTOK, HD // 2], F32)
            sk = cn.tile([NTOK, HD // 2], F32)
            nc.gpsimd.dma_start(cq[:], cosq[:])
            nc.gpsimd.dma_start(sq[:], sinq[:])
            nc.gpsimd.dma_start(ck[:], cosk[:])
            nc.gpsimd.dma_start(sk[:], sink[:])
            penApp_sb = cn.tile([ROWS, NTOK], F32)
            nc.gpsimd.dma_start(penApp_sb[:], penApp[:])
            maskP_sb = cn.tile([ROWS, T], BF16)
            nc.gpsimd.dma_start(maskP_sb[:], maskP[:])

            ident = cn.tile([128, 128], F32)
            make_identity(nc, ident[:])
            ident_bf = cn.tile([128, 128], BF16)
            nc.vector.tensor_copy(ident_bf[:], ident[:])

            qkv_sb = cn.tile([NTOK, CW + 2 * HD], F32)
            qrot = cn.tile([NTOK, CW], F32)
            krot = cn.tile([NTOK, HD], F32)
            qT_sb = cn.tile([128, ROWS], BF16)
            knT_sb = cn.tile([128, NTOK], BF16)
            vnew4 = cn.tile([NTOK, CW], BF16)
            P = cn.tile([ROWS, T + NTOK], BF16)
            PT_sb = cn.tile([128, 16 * 128], BF16)
            PTapp = cn.tile([NTOK, 128], BF16)
            av_app_sb = cn.tile([128, CW], F32)
            den_h = cn.tile([ROWS, 3], F32)
            den = cn.tile([ROWS, 1], F32)
            rec = cn.tile([ROWS, 1], F32)
            attnT = cn.tile([128, ROWS], BF16)
            av_gat = cn.tile([NTOK, CW], F32)
            ga_sb = cn.tile([128, ndc * NTOK], BF16)

            # ---------------- phase A: projections + RoPE -----------------
            with (
                tc.tile_pool(name="psA", bufs=1, space="PSUM") as psA,
            ):
                # PE warmup: dense dummy matmuls so HAM unthrottles to 2.4GHz
                # while the first weight DMAs are still in flight.
                warm_ps = psA.tile([128, 128], F32, tag="warm", space="PSUM")
                for _ in range(48):
                    nc.tensor.matmul(warm_ps[:], ident[:], ident[:],
                                     start=True, stop=True)
                qkv_ps = psA.tile([NTOK, CW + 2 * HD], F32, space="PSUM")
                wq_ts = []
                for pc in range(4):
                    wt = kp.tile([128, 8 * CW], BF16, tag="kt")
                    (nc.sync if pc % 2 == 0 else nc.scalar).dma_start(wt[:], wqF[pc])
                    wq_ts.append(wt)
                wkv_ts = []
                for pc in range(2):
                    wt = vp.tile([128, 16 * 256], BF16, tag="v")
                    (nc.sync if pc % 2 == 0 else nc.scalar).dma_start(wt[:], wkvF[pc])
                    wkv_ts.append(wt)
                for pc in range(4):
                    for cc in range(8):
                        c = pc * 8 + cc
                        nc.tensor.matmul(
                            qkv_ps[:, 0:CW],
                            xT_sb[:, c * NTOK:(c + 1) * NTOK],
                            wq_ts[pc][:, cc * CW:(cc + 1) * CW],
                            start=(c == 0), stop=(c == ndc - 1),
                        )
                for pc in range(2):
                    for cc in range(16):
                        c = pc * 16 + cc
                        nc.tensor.matmul(
                            qkv_ps[:, CW:CW + 256],
                            xT_sb[:, c * NTOK:(c + 1) * NTOK],
                            wkv_ts[pc][:, cc * 256:(cc + 1) * 256],
                            start=(c == 0), stop=(c == ndc - 1),
                        )
                nc.vector.tensor_copy(qkv_sb[:], qkv_ps[:])

                # RoPE: q packed across 4 heads (cos/sin tables pre-tiled and
                # pre-scaled by 1/sqrt(HD)); k single head unscaled.
                tq1 = cn.tile([NTOK, HPC * HD // 2], F32)
                tq2 = cn.tile([NTOK, HPC * HD // 2], F32)

                def rope(src_ap, dst_ap, c_t, s_t, t1, t2):
                    sv = src_ap.rearrange("p (i two) -> p two i", two=2)
                    dv = dst_ap.rearrange("p (i two) -> p two i", two=2)
                    nc.vector.tensor_tensor(t1[:], sv[:, 0, :], c_t[:], op=ALU.mult)
                    nc.vector.tensor_tensor(t2[:], sv[:, 1, :], s_t[:], op=ALU.mult)
                    nc.vector.tensor_tensor(dv[:, 0, :], t1[:], t2[:], op=ALU.subtract)
                    nc.vector.tensor_tensor(t1[:], sv[:, 0, :], s_t[:], op=ALU.mult)
                    nc.vector.tensor_tensor(t2[:], sv[:, 1, :], c_t[:], op=ALU.mult)
                    nc.vector.tensor_tensor(dv[:, 1, :], t1[:], t2[:], op=ALU.add)

                rope(qkv_sb[:, 0:CW], qrot[:], cq, sq, tq1, tq2)
                rope(qkv_sb[:, CW:CW + HD], krot[:],
                     ck, sk, tq1[:, 0:HD // 2], tq2[:, 0:HD // 2])

                # transposes: qT [128, (h, b, s)]; k_new^T [128, (b, s)]
                for h in range(HPC):
                    tp = psA.tile([128, NTOK], F32, tag="tp", bufs=2, space="PSUM")
                    nc.tensor.transpose(tp[:], qrot[:, h * HD:(h + 1) * HD], ident[:NTOK, :NTOK])
                    nc.vector.tensor_copy(qT_sb[:, h * NTOK:(h + 1) * NTOK], tp[:])
                tp = psA.tile([128, NTOK], F32, tag="tp", bufs=2, space="PSUM")
                nc.tensor.transpose(tp[:], krot[:], ident[:NTOK, :NTOK])
                nc.vector.tensor_copy(knT_sb[:], tp[:])

                # v_new tiled 4x across head blocks (GQA repeat), bf16
                for h in range(HPC):
                    nc.vector.tensor_copy(vnew4[:, h * HD:(h + 1) * HD],
                                          qkv_sb[:, CW + HD:CW + 2 * HD])

                # ---- new-position score block + its attn@V contribution --
                app_ps = psA.tile([ROWS, NTOK], F32, tag="app", space="PSUM")
                nc.tensor.matmul(app_ps[:], qT_sb[:], knT_sb[:], start=True, stop=True)
                nc.vector.tensor_tensor(app_ps[:], app_ps[:], penApp_sb[:], op=ALU.add)
                nc.scalar.activation(P[:, T:T + NTOK], app_ps[:], AF.Exp)
                tp2 = psA.tile([NTOK, 128], BF16, tag="tpb", bufs=1, space="PSUM")
                nc.tensor.transpose(tp2[:], P[:, T:T + NTOK], ident_bf[:])
                nc.vector.tensor_copy(PTapp[:], tp2[:])
                av_app_ps = psA.tile([128, CW], F32, tag="avapp", space="PSUM")
                nc.tensor.matmul(av_app_ps[:], PTapp[:], vnew4[:], start=True, stop=True)
                nc.vector.tensor_copy(av_app_sb[:], av_app_ps[:])
                nc.vector.tensor_reduce(den_h[:, 2:3], P[:, T:T + NTOK],
                                        axis=mybir.AxisListType.X, op=ALU.add)

            av_acc = [cn.tile([128, CW], F32, name=f"av_acc{b}") for b in range(B)]

            with tc.tile_pool(name="psM", bufs=1, space="PSUM") as psM:

                def scores_half(half, ktiles):
                    for b in range(B):
                        ktb = ktiles[b]
                        wps = psM.tile([128, 512], F32, tag="sc", bufs=3, space="PSUM")
                        for _ in range(8):
                            nc.tensor.matmul(wps[:, 0:128], ident_bf[:],
                                             ident_bf[:], start=True, stop=True)
                        stg = st.tile([128, TH], BF16, tag="stg")
                        for jj in range(2):
                            sc = psM.tile([128, 512], F32, tag="sc", bufs=3, space="PSUM")
                            for g in range(HPC):
                                r0 = g * NTOK + b * S
                                nc.tensor.matmul(
                                    sc[g * NTOK:g * NTOK + S, :],
                                    qT_sb[:, r0:r0 + S],
                                    ktb[:, g * TH + jj * 512:g * TH + (jj + 1) * 512],
                                    start=True, stop=True,
                                    tile_position=(0, g * NTOK),
                                )
                            nc.scalar.activation(stg[:, jj * 512:(jj + 1) * 512], sc[:], AF.Exp)
                        # scatter the 16 live rows into P
                        for g in range(HPC):
                            r0 = g * NTOK + b * S
                            nc.gpsimd.dma_start(
                                P[r0:r0 + S, half * TH:(half + 1) * TH],
                                stg[g * NTOK:g * NTOK + S, :],
                            )
                    # mask out replaced cache columns, then transpose chunks
                    nc.vector.tensor_tensor(
                        P[:, half * TH:(half + 1) * TH],
                        P[:, half * TH:(half + 1) * TH],
                        maskP_sb[:, half * TH:(half + 1) * TH], op=ALU.mult)
                    for ch in range(half * 8, half * 8 + 8):
                        tp3 = psM.tile([128, 128], BF16, tag="pt", bufs=2, space="PSUM")
                        nc.tensor.transpose(tp3[:], P[:, ch * 128:(ch + 1) * 128], ident_bf[:])
                        nc.vector.tensor_copy(PT_sb[:, ch * 128:(ch + 1) * 128], tp3[:])
                    nc.vector.tensor_reduce(den_h[:, half:half + 1],
                                            P[:, half * TH:(half + 1) * TH],
                                            axis=mybir.AxisListType.X, op=ALU.add)

                def av_half(half, vtiles):
                    wps3 = psM.tile([128, 512], F32, tag="sc", bufs=3, space="PSUM")
                    for _ in range(28):
                        nc.tensor.matmul(wps3[:, 0:128], ident_bf[:],
                                         ident_bf[:], start=True, stop=True)
                    if half == 1:
                        nc.vector.tensor_tensor(den[:], den_h[:, 0:1], den_h[:, 1:2], op=ALU.add)
                        nc.vector.tensor_tensor(den[:], den[:], den_h[:, 2:3], op=ALU.add)
                        nc.vector.reciprocal(rec[:], den[:])
                    for b in range(B):
                        vtb = vtiles[b]
                        wps2 = psM.tile([128, 512], F32, tag="sc", bufs=3, space="PSUM")
                        for _ in range(20):
                            nc.tensor.matmul(wps2[:, 0:128], ident_bf[:],
                                             ident_bf[:], start=True, stop=True)
                        av_ps = psM.tile([128, CW], F32, tag="av", bufs=3, space="PSUM")
                        for ch in range(8):
                            nc.tensor.matmul(
                                av_ps[:],
                                PT_sb[:, (half * 8 + ch) * 128:(half * 8 + ch + 1) * 128],
                                vtb[:, ch * CW:(ch + 1) * CW],
                                start=(ch == 0), stop=(ch == 7),
                            )
                        if half == 0:
                            nc.vector.tensor_copy(av_acc[b][:], av_ps[:])
                        else:
                            avs = avp.tile([128, CW], F32, tag="avs")
                            nc.vector.tensor_tensor(avs[:], av_ps[:], av_acc[b][:], op=ALU.add)
                            nc.vector.tensor_tensor(avs[:], avs[:], av_app_sb[:], op=ALU.add)
                            nc.vector.tensor_scalar_mul(avs[:], avs[:], rec[:])
                            for g in range(HPC):
                                r0 = g * NTOK + b * S
                                nc.sync.dma_start(
                                    av_gat[b * S:(b + 1) * S, g * HD:(g + 1) * HD],
                                    avs[r0:r0 + S, g * HD:(g + 1) * HD],
                                )

                # ---- pipelined halves: K h0 | V h0 + K h1 | V h1 ----------
                def qeng(i):
                    return nc.sync if i % 2 == 0 else nc.scalar

                kt0 = []
                for b in range(B):
                    ktb = kp.tile([128, HPC * TH], BF16, tag="kt")
                    qeng(b).dma_start(ktb[:], kF[b])
                    kt0.append(ktb)
                scores_half(0, kt0)

                vt0, kt1 = [], []
                for b in range(B):
                    vtb = vp.tile([128, 8 * CW], BF16, tag="v")
                    qeng(b).dma_start(vtb[:], vF[b])
                    vt0.append(vtb)
                    ktb = kp.tile([128, HPC * TH], BF16, tag="kt")
                    qeng(b + 1).dma_start(ktb[:], kF[B + b])
                    kt1.append(ktb)
                av_half(0, vt0)
                scores_half(1, kt1)

                vt1 = []
                for b in range(B):
                    vtb = vp.tile([128, 8 * CW], BF16, tag="v")
                    qeng(b).dma_start(vtb[:], vF[B + b])
                    vt1.append(vtb)
                wo_ts = []
                for pc in range(4):
                    wt = kp.tile([128, 8 * CW], BF16, tag="kt")
                    qeng(pc).dma_start(wt[:], woF[:, pc * 8 * CW:(pc + 1) * 8 * CW])
                    wo_ts.append(wt)
                av_half(1, vt1)

                # transpose per head -> attnT [128 d, (h, tok)]
                for g in range(HPC):
                    tpx = psM.tile([128, NTOK], F32, tag="pt", bufs=2, space="PSUM")
                    nc.tensor.transpose(tpx[:], av_gat[:, g * HD:(g + 1) * HD],
                                        ident[:NTOK, :NTOK])
                    nc.vector.tensor_copy(attnT[:, g * NTOK:(g + 1) * NTOK], tpx[:])

                # ---------------- AllGather ------------------------------
                bounce_in = dr.tile([HPC, 128, NTOK], BF16)
                gathered = dr.tile([ndc, 128, NTOK], BF16)
                for g in range(HPC):
                    nc.sync.dma_start(
                        bounce_in[g],
                        attnT[:, g * NTOK:(g + 1) * NTOK],
                    )
                nc.gpsimd.collective_compute(
                    "AllGather",
                    ALU.bypass,
                    replica_groups=[list(range(N_CORES))],
                    ins=[bounce_in[:].opt()],
                    outs=[gathered[:].opt()],
                )
                for q in range(4):
                    (nc.sync if q % 2 == 0 else nc.scalar).dma_start(
                        ga_sb[:, q * 8 * NTOK:(q + 1) * 8 * NTOK]
                        .rearrange("p (c t) -> p c t", t=NTOK),
                        gathered[q * 8:(q + 1) * 8].rearrange("c p t -> p c t"),
                    )

                # ---------------- output projection -----------------------
                y_ps = psM.tile([NTOK, CW], F32, tag="av", bufs=3, space="PSUM")
                for c in range(ndc):
                    nc.tensor.matmul(
                        y_ps[:],
                        ga_sb[:, c * NTOK:(c + 1) * NTOK],
                        wo_ts[c // 8][:, (c % 8) * CW:(c % 8 + 1) * CW],
                        start=(c == 0), stop=(c == ndc - 1),
                    )
                y_sb = cn.tile([NTOK, CW], F32)
                nc.vector.tensor_copy(y_sb[:], y_ps[:])
                nc.sync.dma_start(out[:], y_sb[:])

                if DEBUG_TAPS:
                    taps = {
                        "d_qkv": qkv_sb, "d_qrot": qrot, "d_krot": krot,
                        "d_qT": qT_sb, "d_knT": knT_sb, "d_vnew4": vnew4,
                        "d_P": P, "d_PT": PT_sb, "d_den": den_h,
                        "d_rec": rec, "d_avgat": av_gat, "d_attnT": attnT,
                        "d_ga": ga_sb, "d_avapp": av_app_sb,
                    }
                    for nm, t in taps.items():
                        ap = t[:]
                        dt_out = nc.declare_dram_parameter(
                            nm, list(ap.shape), ap.dtype, isOutput=True)
                        nc.sync.dma_start(dt_out[:], ap)

    nc.compile()
    return nc


def _get_nc():
    if "nc" not in _CACHE:
        _CACHE["nc"] = _build()
    return _CACHE["nc"]


def _bf16(a):
    return np.ascontiguousarray(a).astype(ml_dtypes.bfloat16)


def _prep_in_maps(x, start_pos, angles, cache_k, cache_v, wq, wk, wv, wo, layer_idx):
    li = int(layer_idx)
    ndc = DIM // 128
    x = np.asarray(x, np.float32).reshape(NTOK, DIM)
    # xT flat: [128 p][c 32][t 32] = x[t, c*128+p]
    xT_f = _bf16(x.reshape(NTOK, ndc, 128).transpose(2, 1, 0).reshape(128, ndc * NTOK))

    ang = np.asarray(angles, np.float64).reshape(NTOK, HD // 2)
    alpha = 1.0 / np.sqrt(HD)
    cq1 = (np.cos(ang) * alpha).astype(np.float32)
    sq1 = (np.sin(ang) * alpha).astype(np.float32)
    cosq_f = np.tile(cq1, (1, HPC))
    sinq_f = np.tile(sq1, (1, HPC))
    cosk_f = np.cos(ang).astype(np.float32)
    sink_f = np.sin(ang).astype(np.float32)
    sp = np.asarray(start_pos).astype(np.int64)

    # rows r = g*32 + b*4 + s; maskP kills the replaced cache cols per-batch
    maskP_f = np.ones((ROWS, T), np.float32)
    penApp_f = np.full((ROWS, NTOK), NEG, np.float32)
    for r in range(ROWS):
        b = (r % NTOK) // S
        maskP_f[r, sp[b]:sp[b] + S] = 0.0
        penApp_f[r, b * S:(b + 1) * S] = 0.0
    maskP_f = maskP_f.astype(ml_dtypes.bfloat16)

    wq = np.asarray(wq, np.float32)
    wk = np.asarray(wk, np.float32)
    wv = np.asarray(wv, np.float32)
    wo = np.asarray(wo, np.float32)
    ck_l = np.asarray(cache_k, np.float32)[:, :, li, :]
    cv_l = np.asarray(cache_v, np.float32)[:, :, li, :]

    in_maps = []
    for c in range(N_CORES):
        qs, qe = c * CW, (c + 1) * CW
        ks, ke = c * HD, (c + 1) * HD
        # wq flat: [pc 4][128 p][cc 8][cw 512] = wq[qs+cw, (pc*8+cc)*128+p]
        wqT = wq[qs:qe, :].T                      # [DIM, CW]
        wq_f = _bf16(wqT.reshape(ndc, 128, CW).reshape(4, 8, 128, CW)
                     .transpose(0, 2, 1, 3).reshape(4, 128, 8 * CW))
        # wkv flat: [pc 2][128 p][cc 16][256] ; 256 = k|v cols
        wkvT = np.concatenate([wk[ks:ke].T, wv[ks:ke].T], axis=1)  # [DIM, 256]
        wkv_f = _bf16(wkvT.reshape(ndc, 128, 256).reshape(2, 16, 128, 256)
                      .transpose(0, 2, 1, 3).reshape(2, 128, 16 * 256))
        # wo flat: [128 p][g 32][cw 512] = wo[qs+cw, g*128+p]
        woT = wo[qs:qe, :].T                      # [DIM, CW]
        wo_f = _bf16(woT.reshape(ndc, 128, CW).transpose(1, 0, 2).reshape(128, ndc * CW))
        # K flat: [(half b) 16][128 p=d][g 4][t' 1024] = K[b, half*TH+t', qs+g*128+p]
        kc = ck_l[:, :, qs:qe].reshape(B, 2, TH, HPC, 128)
        k_f = _bf16(kc.transpose(1, 0, 4, 3, 2).reshape(2 * B, 128, HPC * TH))
        # V flat: [(half b) 16][128 p][ch 8][cw 512] = V[b, half*TH+ch*128+p, qs+cw]
        vc = cv_l[:, :, qs:qe].reshape(B, 2, 8, 128, CW)
        v_f = _bf16(vc.transpose(1, 0, 3, 2, 4).reshape(2 * B, 128, 8 * CW))
        in_maps.append({
            "xT": xT_f,
            "wqF": wq_f, "wkvF": wkv_f, "woF": wo_f,
            "kF": k_f, "vF": v_f,
            "cosq": cosq_f, "sinq": sinq_f, "cosk": cosk_f, "sink": sink_f,
            "maskP": maskP_f, "penApp": penApp_f,
        })
    return in_maps


def kernel(x, start_pos, angles, cache_k, cache_v, mask, wq, wk, wv, wo, layer_idx):
    del mask  # zeros by construction
    in_maps = _prep_in_maps(x, start_pos, angles, cache_k, cache_v, wq, wk, wv, wo, layer_idx)
    nc = _get_nc()
    res = run_bass_kernel_spmd(nc, in_maps, core_ids=list(range(N_CORES)))
    _CACHE["last_result"] = res
    y = np.concatenate([res.results[c]["out"] for c in range(N_CORES)], axis=1)
    return y.reshape(B, S, DIM)
